# revision 1
# baseline (speedup 1.0000x reference)
"""MoE BaseLayer kernel for Trainium2 (8 NeuronCores, expert parallelism).

Strategy (per the expert-parallelism sharding hint):
  * Host computes token->expert assignment (scores = x @ centroids.T, argmax)
    -- this IS the shard function: tokens are dispatched to the core owning
    their expert (the host-side equivalent of the All2All in the original),
    and the gate alpha = sigmoid(score of the assigned expert) falls out of
    the same routing scores. The dispatch/combine packing also applies
    LayerNorm (ln_g/ln_b folded into W1/b1 exactly; stats in f32 identical
    to the reference) and the final alpha-blend + residual -- together
    ~0.1% of the FLOPs. The device runs the expert FFN, 99.9% of the work.
  * Core e holds expert e's weights only and runs FF1 -> ReLU -> FF2 over
    its C routed tokens (C = ceil(max_count/8)*8), returning ffn^T.
  * Host combines: y[t] = x[t] + alpha[t] * (ffn[t] + b2[e]), scattered
    back to original token order.

Device kernel (per core), v5 tuned from NTFF traces:
  * single DMA queue in strict consumption order:
    head(xhat^T + w1[ft 0-1]) | b1 | w1[ft 2-3] | w2q0 | w1g1 | w2q1 | ...
    per-core HBM read bandwidth measured ~245 GB/s makes arrival order the
    schedule; the FF1 critical prefix rides one merged 543KB transfer
    (fewer, larger transfers win -- splitting it measurably regressed).
    All input DMAs issue from the SCALAR engine, whose preamble retires
    ~1us before sync's, starting the stream earlier.
  * no ACT-engine work at all (ReLU and the output casts run on DVE), so
    the 1.5us ACT table load disappears from the scalar queue.
  * all matmul operands bf16 (half DMA, 1 cyc/row); f32 PSUM accumulate;
    rel-err 2.2e-3 vs the 2e-2 gate.
  * no warm-up spin: three separate attempts showed the HAM duty-cycler
    cannot be reliably gamed -- spins either spend the fixed ~17us
    full-rate grant early or queue ahead of the real matmuls. Measured:
    ~0.7-1.0 rows/ns before the grant triggers, ~2.35 rows/ns (full 2.4GHz,
    119ns per 280-row matmul, zero bubbles) inside it.
  * FF1 w1-stationary streams C tokens per (ft,kt); ReLU+b1 is one DVE
    tensor_scalar (add, max) off the f32 PSUM accumulator
  * FF2 token-streaming: w2 128x128 blocks stationary, h streams ->
    y^T strips [128(D), C] accumulate in PSUM (16*4 matmuls of C rows,
    ~30% fewer PE rows than streaming w2)
  * ffn^T leaves as bf16 via one merged 287KB transfer (four DVE casts
    into one flat tile, a single DMA issue)
"""

import numpy as np
import ml_dtypes

BF16 = ml_dtypes.bfloat16

E, D, F = 8, 512, 2048
LN_EPS = 1e-5
P = 128

_CACHE = {}


def _build(C):
    import concourse.tile as tile
    from concourse import bacc, mybir

    f32 = mybir.dt.float32
    bf = mybir.dt.bfloat16
    ACT = mybir.ActivationFunctionType
    NT = -(-C // P)
    KT = D // P                       # 4
    FT = F // P                       # 16
    assert NT <= 4, "single-group kernel (C <= 512)"

    nc = bacc.Bacc("TRN2", target_bir_lowering=False, num_devices=E)
    head_d = nc.dram_tensor("head", [P, KT * C + 1024], bf,
                            kind="ExternalInput")
    wall_d = nc.dram_tensor("wall", [2 * (FT // 4), P, KT * 512], bf,
                            kind="ExternalInput")
    hm_d = nc.dram_tensor("hm", [P, FT], f32, kind="ExternalInput")
    y_d = nc.dram_tensor("y", [P, KT * C], bf, kind="ExternalOutput")

    with tile.TileContext(nc) as tc:
        with (
            tc.tile_pool(name="wpool", bufs=1) as wpool,
            tc.tile_pool(name="xpool", bufs=1) as xpool,
            tc.tile_pool(name="hpool", bufs=3) as hpool,
            tc.tile_pool(name="pacc", bufs=2, space="PSUM") as pacc,
            tc.tile_pool(name="pyt", bufs=1, space="PSUM") as pyt,
        ):
            # ---- input DMA stream: one queue, strict consumption order ----
            # w1 layout (host-packed): col = (ft%4)*512 + kt*128 + f%128.
            # The FF1 critical prefix -- xhat^T plus w1[ft 0-1] -- rides ONE
            # 543KB transfer (the DMA engine rewards fewer, larger
            # transfers; splitting this prefix measurably regressed).
            head_t = xpool.tile([P, KT * C + 1024], bf, name="head_t",
                                tag="head_t")
            nc.scalar.dma_start(out=head_t, in_=head_d[:])
            w1g = [None] * (FT // 4)
            w2q = [None] * (FT // 4)

            hm_t = xpool.tile([P, FT], f32, name="hm_t", tag="hm_t")
            nc.scalar.dma_start(out=hm_t, in_=hm_d[:])
            w1g0b = wpool.tile([P, 1024], bf, name="w1g0b", tag="w1g0b")
            nc.scalar.dma_start(out=w1g0b, in_=wall_d[0][:, 1024:2048])

            def load_w1g(g):
                t = wpool.tile([P, KT * 512], bf, name=f"w1g{g}", tag=f"w1g{g}")
                nc.scalar.dma_start(out=t, in_=wall_d[2 * g])
                w1g[g] = t

            def load_w2q(g):
                t = wpool.tile([P, 4 * D], bf, name=f"w2q{g}", tag=f"w2q{g}")
                nc.scalar.dma_start(out=t, in_=wall_d[2 * g + 1])
                w2q[g] = t

            load_w2q(0)
            for g in range(1, FT // 4):
                load_w1g(g)
                load_w2q(g)

            b1T = hm_t

            # ---- FF1 + FF2 (token-streaming), pipelined one F-tile apart --
            yT = [
                pyt.tile([P, C], f32, name=f"yt{dt}", tag=f"yt{dt}")
                for dt in range(KT)
            ]
            hs = [None] * FT

            def ff1(ft):
                acc = pacc.tile([P, C], f32, name="acc1", tag="acc1")
                for kt in range(KT):
                    c0 = (ft % 4) * 512 + kt * P
                    if ft < 2:
                        lhsT = head_t[:, KT * C + c0:KT * C + c0 + P]
                    elif ft < 4:
                        lhsT = w1g0b[:, c0 - 1024:c0 - 1024 + P]
                    else:
                        lhsT = w1g[ft // 4][:, c0:c0 + P]
                    nc.tensor.matmul(
                        acc, lhsT, head_t[:, kt * C:(kt + 1) * C],
                        start=(kt == 0), stop=(kt == KT - 1),
                    )
                h = hpool.tile([P, C], bf, name="h", tag="h")
                nc.vector.tensor_scalar(
                    out=h, in0=acc,
                    scalar1=b1T[:, ft:ft + 1], scalar2=0.0,
                    op0=mybir.AluOpType.add, op1=mybir.AluOpType.max,
                )
                hs[ft] = h

            def ff2(ft):
                for dt in range(KT):
                    c0 = (ft % 4) * D + dt * P
                    lhsT = w2q[ft // 4][:, c0:c0 + P]
                    nc.tensor.matmul(
                        yT[dt], lhsT, hs[ft][:],
                        start=(ft == 0), stop=(ft == FT - 1),
                    )

            ff1(0)
            for ft in range(1, FT):
                ff1(ft)
                ff2(ft - 1)
            ff2(FT - 1)

            # ---- ffn^T -> SBUF (bf16) -> one merged DRAM transfer ---------
            yTs = hpool.tile([P, KT * C], bf, name="yTs", tag="yTs")
            for dt in range(KT):
                nc.vector.tensor_copy(
                    out=yTs[:, dt * C:(dt + 1) * C], in_=yT[dt][:, :C]
                )
            nc.sync.dma_start(out=y_d[:], in_=yTs)

    nc.compile()
    return nc


def _get_nc(C):
    if C not in _CACHE:
        _CACHE[C] = _build(C)
    return _CACHE[C]


def _route(feats, centroids):
    """Token->expert assignment + gate, computed the same way the reference
    does (jax on CPU) so argmax near-ties resolve identically."""
    try:
        import jax
        import jax.numpy as jnp

        with jax.default_device(jax.devices("cpu")[0]):
            scores = jnp.asarray(feats) @ jnp.asarray(centroids).T
            assign = jnp.argmax(scores, axis=1)
            alpha = jax.nn.sigmoid(
                jnp.take_along_axis(scores, assign[:, None], axis=1)
            )
            return np.asarray(assign), np.asarray(alpha, dtype=np.float32)
    except Exception:
        scores = feats @ centroids.T
        assign = np.argmax(scores, axis=1)
        alpha = 1.0 / (1.0 + np.exp(-scores[np.arange(len(assign)), assign]))
        return assign, alpha[:, None].astype(np.float32)


def prepare(x, centroids, ln_g, ln_b, W1, b1, W2, b2):
    """Shard the full inputs: route tokens to experts, apply LayerNorm while
    packing (stats in f32, identical to the reference), build per-core input
    maps. Returns (C, in_maps, routing_state)."""
    x = np.asarray(x)
    orig_shape = x.shape
    feats = np.ascontiguousarray(x.reshape(-1, D), dtype=np.float32)
    centroids = np.asarray(centroids, dtype=np.float32)

    assign, alpha = _route(feats, centroids)

    idx = [np.nonzero(assign == e)[0] for e in range(E)]
    max_count = max(len(ix) for ix in idx)
    C = max(128, -(-max_count // 8) * 8)

    W1 = np.asarray(W1, dtype=np.float32)
    W2 = np.asarray(W2, dtype=np.float32)
    b1 = np.asarray(b1, dtype=np.float32)
    b2 = np.asarray(b2, dtype=np.float32)
    ln_g = np.asarray(ln_g, dtype=np.float32)
    ln_b = np.asarray(ln_b, dtype=np.float32)

    KT = D // P
    FT = F // P
    in_maps = []
    for e in range(E):
        NT = -(-C // P)
        xs = np.zeros((NT * P, D), dtype=np.float32)
        xs[: len(idx[e])] = feats[idx[e]]
        # fold LN affine into the first FFN layer (exact reparameterization)
        w1_eff = ln_g[e][:, None] * W1[e]
        b1_eff = ln_b[e] @ W1[e] + b1[e]

        # LayerNorm (f32 stats, like the reference), shipped pre-transposed
        mu = xs.mean(axis=1, keepdims=True)
        var = xs.var(axis=1, keepdims=True)
        xh = ((xs - mu) / np.sqrt(var + LN_EPS)).astype(BF16)[:C]
        hxt = np.ascontiguousarray(
            xh.T.reshape(KT, P, C).transpose(1, 0, 2).reshape(P, KT * C)
        )

        hm = np.ascontiguousarray(b1_eff.reshape(FT, P).T)

        wall = np.empty((2 * (FT // 4), P, KT * 512), dtype=BF16)
        for g in range(FT // 4):
            # w1: col = (ft%4)*512 + kt*128 + f%128
            wall[2 * g] = (
                w1_eff[:, g * 512:(g + 1) * 512]
                .reshape(KT, P, 4, P).transpose(1, 2, 0, 3).reshape(P, KT * 512)
            )
            wall[2 * g + 1] = (
                W2[e][4 * g * P:(4 * g + 4) * P, :]
                .reshape(4, P, D).transpose(1, 0, 2).reshape(P, 4 * D)
            )
        head = np.concatenate([hxt, wall[0][:, 0:1024]], axis=1)
        in_maps.append(dict(head=np.ascontiguousarray(head), wall=wall, hm=hm))
    return C, in_maps, (idx, alpha, feats, b2, orig_shape)


def kernel(x, centroids, ln_g, ln_b, W1, b1, W2, b2):
    from concourse.bass_utils import run_bass_kernel_spmd

    C, in_maps, (idx, alpha, feats, b2v, orig_shape) = prepare(
        x, centroids, ln_g, ln_b, W1, b1, W2, b2
    )
    nc = _get_nc(C)
    res = run_bass_kernel_spmd(nc, in_maps, core_ids=list(range(E)))

    T = int(np.prod(orig_shape[:-1]))
    out = np.empty((T, D), dtype=np.float32)
    for e in range(E):
        n = len(idx[e])
        ffn = (
            res.results[e]["y"].reshape(P, D // P, C).transpose(1, 0, 2)
            .reshape(D, C).T[:n].astype(np.float32)
        )
        out[idx[e]] = feats[idx[e]] + alpha[idx[e]] * (ffn + b2v[e])
    return out.reshape(orig_shape)



# revision 17
# speedup vs baseline: 1.0773x; 1.0773x over previous
"""MoE BaseLayer kernel for Trainium2 (8 NeuronCores, expert parallelism).

Strategy (per the expert-parallelism sharding hint):
  * Host computes token->expert assignment (scores = x @ centroids.T, argmax)
    -- this IS the shard function: tokens are dispatched to the core owning
    their expert (the host-side equivalent of the All2All in the original),
    and the gate alpha = sigmoid(score of the assigned expert) falls out of
    the same routing scores. The dispatch/combine packing also applies
    LayerNorm (ln_g/ln_b folded into W1/b1 exactly; stats in f32 identical
    to the reference) and the final alpha-blend + residual -- together
    ~0.1% of the FLOPs. The device runs the expert FFN, 99.9% of the work.
  * Core e holds expert e's weights only and runs FF1 -> ReLU -> FF2 over
    its C routed tokens (C = ceil(max_count/8)*8), returning ffn^T.
  * Host combines: y[t] = x[t] + alpha[t] * (ffn[t] + b2[e]), scattered
    back to original token order.

Device kernel v6.1 (per core), tuned from NTFF traces of v5 (37.6us):
  * weights shipped as fp8 e3m4 (x64 power-of-2 scale; undone in the output
    cast): halves weight DMA (4MB -> 2MB). Activations stay bf16; matmuls
    are mixed-dtype (e3m4 lhsT, bf16 rhs). Measured rel-err 1.35e-2 vs the
    2e-2 gate (bf16 was 2.2e-3; any e4m3 operand pushes past the gate).
  * 10 warm-up matmuls on a zeroed tile fill the otherwise-dead DMA-wait
    window (t~7.6-12us): the HAM full-rate grant triggers on ~4us of
    CONTINUOUS tensor activity (gaps reset it; cold ~0.8 cols/ns, granted
    ~2.7 cols/ns, grant lasts as long as needed), so the grant fires while
    the dummies run and the real 128-matmul stream executes entirely at
    full rate (~15.3us, zero bubbles). Shorter warm-ups lose the race to
    bridge until the head DMA lands on the slowest core and regress 3-5us.
  * single input DMA queue (scalar), wide-row transfers only (small-row
    transfers crawl on the still-ramping DMA engine: ~150->400 B/ns over
    ~6us), in strict consumption order; w2[dt0] is pulled early because
    the tile scheduler interleaves FF2-dt0 matmuls among FF1 as h strips
    land, and a late w2 DMA head-of-line-blocks the tensor queue.
  * FF1 w1-stationary streams C tokens per (ft,kt); ReLU+b1 is one DVE
    tensor_scalar (add, max) off the f32 PSUM accumulator into a resident
    bf16 h strip (16 strips, 10KB/partition).
  * FF2 runs dt-major so each 128-row output strip finishes early; its
    cast + 80KB DMA overlap the next strip's matmuls, leaving only the
    last strip's drain exposed.
"""

import os

import numpy as np
import ml_dtypes

BF16 = ml_dtypes.bfloat16
E3M4 = ml_dtypes.float8_e3m4

E, D, F = 8, 512, 2048
LN_EPS = 1e-5
P = 128

# (weights dtype, warmup matmul count); overridable for experiments
_CFG = os.environ.get("BASS_V6_CFG", "e3m4:10")

_CACHE = {}

W_SCALE = 64.0  # power-of-2 scale for e3m4 weights (w1 and w2)


def _build(C, cfg):
    import concourse.tile as tile
    from concourse import bacc, mybir

    w_dt_name, nwarm = cfg.split(":")
    nwarm = int(nwarm)

    f32 = mybir.dt.float32
    bf = mybir.dt.bfloat16
    WDT = mybir.dt.float8e3 if w_dt_name == "e3m4" else bf
    out_scale = (1.0 / (W_SCALE * W_SCALE)) if w_dt_name == "e3m4" else 1.0
    KT = D // P                       # 4
    FT = F // P                       # 16
    NT = -(-C // P)
    assert NT <= 4, "single-group kernel (C <= 512)"

    nc = bacc.Bacc("TRN2", target_bir_lowering=False, num_devices=E)
    xh_d = nc.dram_tensor("xh", [P, KT * C], bf, kind="ExternalInput")
    hm_d = nc.dram_tensor("hm", [P, FT], f32, kind="ExternalInput")
    w1a_d = nc.dram_tensor("w1a", [P, 512], WDT, kind="ExternalInput")
    w1b_d = nc.dram_tensor("w1b", [P, 1536], WDT, kind="ExternalInput")
    w1g_d = nc.dram_tensor("w1g", [3, P, 2048], WDT, kind="ExternalInput")
    w2_d = nc.dram_tensor("w2", [KT, P, FT * P], WDT, kind="ExternalInput")
    y_d = nc.dram_tensor("y", [P, KT * C], bf, kind="ExternalOutput")

    with tile.TileContext(nc) as tc:
        with (
            tc.tile_pool(name="wpool", bufs=1) as wpool,
            tc.tile_pool(name="xpool", bufs=1) as xpool,
            tc.tile_pool(name="hpool", bufs=1) as hpool,
            tc.tile_pool(name="opool", bufs=2) as opool,
            tc.tile_pool(name="pacc", bufs=2, space="PSUM") as pacc,
            tc.tile_pool(name="pyt", bufs=2, space="PSUM") as pyt,
            tc.tile_pool(name="pwu", bufs=1, space="PSUM") as pwu,
        ):
            # ---- tensor-engine warm-up during the DMA dead window --------
            if nwarm:
                zer = xpool.tile([P, 512], bf, name="zer", tag="zer")
                nc.gpsimd.memset(zer[:], 0.0)
                wacc = pwu.tile([P, 512], f32, name="wacc", tag="wacc")
                for i in range(nwarm):
                    nc.tensor.matmul(
                        wacc, zer[:, 0:P], zer[:],
                        start=(i == 0), stop=(i == nwarm - 1),
                    )

            # ---- input DMA stream: one queue, strict consumption order ---
            w1a_t = wpool.tile([P, 512], WDT, name="w1a_t", tag="w1a_t")
            nc.scalar.dma_start(out=w1a_t, in_=w1a_d[:])
            xh_t = xpool.tile([P, KT * C], bf, name="xh_t", tag="xh_t")
            nc.scalar.dma_start(out=xh_t, in_=xh_d[:])
            hm_t = xpool.tile([P, FT], f32, name="hm_t", tag="hm_t")
            nc.scalar.dma_start(out=hm_t, in_=hm_d[:])
            w1b_t = wpool.tile([P, 1536], WDT, name="w1b_t", tag="w1b_t")
            nc.scalar.dma_start(out=w1b_t, in_=w1b_d[:])
            w2_t = [None] * KT

            def load_w2(dt):
                t = wpool.tile([P, FT * P], WDT, name=f"w2d{dt}", tag=f"w2d{dt}")
                nc.scalar.dma_start(out=t, in_=w2_d[dt])
                w2_t[dt] = t

            load_w2(0)
            w1g_t = []
            for g in range(3):
                t = wpool.tile([P, 2048], WDT, name=f"w1g{g}", tag=f"w1g{g}")
                nc.scalar.dma_start(out=t, in_=w1g_d[g])
                w1g_t.append(t)
            for dt in range(1, KT):
                load_w2(dt)

            # ---- FF1: h[ft] = relu(acc + b1), bf16 strips stay resident --
            hbuf = hpool.tile([P, FT * C], bf, name="hbuf", tag="hbuf")

            def w1_slice(ft, kt):
                if ft == 0:
                    return w1a_t[:, kt * P:(kt + 1) * P]
                if ft < 4:
                    c0 = (ft - 1) * 512 + kt * P
                    return w1b_t[:, c0:c0 + P]
                c0 = ((ft - 4) % 4) * 512 + kt * P
                return w1g_t[(ft - 4) // 4][:, c0:c0 + P]

            for ft in range(FT):
                acc = pacc.tile([P, C], f32, name="acc1", tag="acc1")
                for kt in range(KT):
                    nc.tensor.matmul(
                        acc, w1_slice(ft, kt), xh_t[:, kt * C:(kt + 1) * C],
                        start=(kt == 0), stop=(kt == KT - 1),
                    )
                nc.vector.tensor_scalar(
                    out=hbuf[:, ft * C:(ft + 1) * C], in0=acc,
                    scalar1=hm_t[:, ft:ft + 1], scalar2=0.0,
                    op0=mybir.AluOpType.add, op1=mybir.AluOpType.max,
                )

            # ---- FF2 dt-major: finish each output strip early, stream it -
            for dt in range(KT):
                yt = pyt.tile([P, C], f32, name=f"yt{dt}", tag="yt")
                for ft in range(FT):
                    nc.tensor.matmul(
                        yt, w2_t[dt][:, ft * P:(ft + 1) * P],
                        hbuf[:, ft * C:(ft + 1) * C],
                        start=(ft == 0), stop=(ft == FT - 1),
                    )
                ys = opool.tile([P, C], bf, name=f"ys{dt}", tag="ys")
                nc.vector.tensor_scalar_mul(ys, yt, out_scale)
                nc.sync.dma_start(out=y_d[:, dt * C:(dt + 1) * C], in_=ys)

    nc.compile()
    return nc


def _get_nc(C, cfg=None):
    key = (C, cfg or _CFG)
    if key not in _CACHE:
        _CACHE[key] = _build(C, cfg or _CFG)
    return _CACHE[key]


def _route(feats, centroids):
    """Token->expert assignment + gate, computed the same way the reference
    does (jax on CPU) so argmax near-ties resolve identically."""
    try:
        import jax
        import jax.numpy as jnp

        with jax.default_device(jax.devices("cpu")[0]):
            scores = jnp.asarray(feats) @ jnp.asarray(centroids).T
            assign = jnp.argmax(scores, axis=1)
            alpha = jax.nn.sigmoid(
                jnp.take_along_axis(scores, assign[:, None], axis=1)
            )
            return np.asarray(assign), np.asarray(alpha, dtype=np.float32)
    except Exception:
        scores = feats @ centroids.T
        assign = np.argmax(scores, axis=1)
        alpha = 1.0 / (1.0 + np.exp(-scores[np.arange(len(assign)), assign]))
        return assign, alpha[:, None].astype(np.float32)


def prepare(x, centroids, ln_g, ln_b, W1, b1, W2, b2, cfg=None):
    """Shard the full inputs: route tokens to experts, apply LayerNorm while
    packing (stats in f32, identical to the reference), build per-core input
    maps. Returns (C, in_maps, routing_state)."""
    cfg = cfg or _CFG
    w_dt_name = cfg.split(":")[0]
    wnp = E3M4 if w_dt_name == "e3m4" else BF16
    ws = W_SCALE if w_dt_name == "e3m4" else 1.0

    x = np.asarray(x)
    orig_shape = x.shape
    feats = np.ascontiguousarray(x.reshape(-1, D), dtype=np.float32)
    centroids = np.asarray(centroids, dtype=np.float32)

    assign, alpha = _route(feats, centroids)

    idx = [np.nonzero(assign == e)[0] for e in range(E)]
    max_count = max(len(ix) for ix in idx)
    C = max(128, -(-max_count // 8) * 8)

    W1 = np.asarray(W1, dtype=np.float32)
    W2 = np.asarray(W2, dtype=np.float32)
    b1 = np.asarray(b1, dtype=np.float32)
    b2 = np.asarray(b2, dtype=np.float32)
    ln_g = np.asarray(ln_g, dtype=np.float32)
    ln_b = np.asarray(ln_b, dtype=np.float32)

    KT = D // P
    FT = F // P
    NT = -(-C // P)
    in_maps = []
    for e in range(E):
        xs = np.zeros((NT * P, D), dtype=np.float32)
        xs[: len(idx[e])] = feats[idx[e]]
        # fold LN affine into the first FFN layer (exact reparameterization)
        w1_eff = ln_g[e][:, None] * W1[e]
        b1_eff = ln_b[e] @ W1[e] + b1[e]

        # LayerNorm (f32 stats, like the reference), shipped pre-transposed
        mu = xs.mean(axis=1, keepdims=True)
        var = xs.var(axis=1, keepdims=True)
        xh = ((xs - mu) / np.sqrt(var + LN_EPS)).astype(BF16)[:C]
        xht = np.ascontiguousarray(
            xh.T.reshape(KT, P, C).transpose(1, 0, 2).reshape(P, KT * C)
        )

        hm = np.ascontiguousarray((ws * b1_eff).reshape(FT, P).T)

        # w1 cols: ft*512 + kt*128 + f%128, partition = k%128
        w1_all = (
            (ws * w1_eff).astype(wnp)
            .reshape(KT, P, FT, P).transpose(1, 2, 0, 3).reshape(P, FT * 512)
        )
        # w2 per dt: cols ft*128 + d%128, partition = f%128
        w2_all = np.ascontiguousarray(
            (ws * W2[e]).astype(wnp)
            .reshape(FT, P, KT, P).transpose(2, 1, 0, 3).reshape(KT, P, FT * P)
        )
        in_maps.append(dict(
            xh=xht,
            hm=hm,
            w1a=np.ascontiguousarray(w1_all[:, :512]),
            w1b=np.ascontiguousarray(w1_all[:, 512:2048]),
            w1g=np.ascontiguousarray(
                w1_all[:, 2048:].reshape(P, 3, 2048).transpose(1, 0, 2)
            ),
            w2=w2_all,
        ))
    return C, in_maps, (idx, alpha, feats, b2, orig_shape)


def kernel(x, centroids, ln_g, ln_b, W1, b1, W2, b2):
    from concourse.bass_utils import run_bass_kernel_spmd

    C, in_maps, (idx, alpha, feats, b2v, orig_shape) = prepare(
        x, centroids, ln_g, ln_b, W1, b1, W2, b2
    )
    nc = _get_nc(C)
    res = run_bass_kernel_spmd(nc, in_maps, core_ids=list(range(E)))

    T = int(np.prod(orig_shape[:-1]))
    out = np.empty((T, D), dtype=np.float32)
    for e in range(E):
        n = len(idx[e])
        ffn = (
            res.results[e]["y"].reshape(P, D // P, C).transpose(1, 0, 2)
            .reshape(D, C).T[:n].astype(np.float32)
        )
        out[idx[e]] = feats[idx[e]] + alpha[idx[e]] * (ffn + b2v[e])
    return out.reshape(orig_shape)
